# revision 23
# baseline (speedup 1.0000x reference)
"""Trainium2 Bass kernel for nn_Attention (dense transformer MHA block).

Reference computation (per batch element b of 8):
    qkv = x @ w_qkv;  q,k,v split into 16 heads of dim 64
    out = softmax(q k^T / 8) v  (per head),  y = out @ w_proj + b_proj

Sharding: pure data-parallel over the batch (B=8 == n_cores). Each core
computes one batch element's full attention with replicated weights; no
collectives. Full inputs in, full outputs out; gather = np.stack.

Per-core dataflow (fp32 weights/x DMA'd directly as f32r -- no convert
copies):
  phase A: per n-tile (2-ahead lookahead), load x, PE-transpose into one
    big xT [d, n] tile (DVE grouped eviction); v-pass matmuls
    (xT-stationary, w_v-moving) packed into v_aug [n, 16*(64+1)] bf16
    tiles with a ones column per head (integrated softmax denominator);
    v_aug evicted on Act. qk matmuls for pair 0 woven into the last two
    v-passes.
  phase B: software-pipelined over heads. Per head-step h: emit S(h+1)
    per j-tile (kT-stationary, qT-moving, K=64) + exp on Act
    (PSUM->SBUF, bf16 out), woven with PV(h) accumulation steps
    (v_aug-stationary, P_T-moving) and, on even steps, the next pair's
    qk matmuls (w-stationary, xT-moving) + DVE evictions. Row 64 of PV
    = softmax denominator -> DVE reciprocal -> Pool partition_broadcast
    -> DVE multiply into attn_T [c, n] (f32r). Projection partial
    accumulations for the first output tiles are woven into the last
    head-step.
  proj: attn_T-stationary @ w_proj-moving; bias added during the
    PSUM->SBUF eviction via a pre-broadcast bias tile (DVE add).
"""

import numpy as np
from contextlib import ExitStack

import concourse.bass as bass
import concourse.bacc as bacc
import concourse.mybir as mybir
from concourse import tile
from concourse.bass_utils import run_bass_kernel_spmd
from concourse.masks import make_identity

F32 = mybir.dt.float32
F32R = mybir.dt.float32r
BF16 = mybir.dt.bfloat16
EXPF = mybir.ActivationFunctionType.Exp

MD = F32R        # matmul operand view dtype for fp32 data
VD = BF16        # v_aug / P_T dtype (PV matmul operands)

CONFIG = {
    "s_bufs": 2,      # S psum tiles in flight ([P, 1024] = 2 banks each)
    "mm_bufs": 4,     # shared matmul-out PSUM pool (1 bank each)
    "p_bufs": 8,      # P_T sbuf tiles (2 heads in flight x 8 j-tiles... bf16)
}

N = 1024          # sequence length (per core)
D = 1024          # model dim
H = 16            # heads
HD = 64           # head dim
SCALE = HD ** -0.5
P = 128           # partitions
NT = N // P       # 8 n-tiles
DT = D // P       # 8 d-chunks
NCORES = 8
NPROJ_WEAVE = 2   # proj output groups partially accumulated in last head-step


def _build(tc, nc, x_d, wqkv_d, wproj_d, bproj_d, y_d, phases="full"):
    mul = mybir.AluOpType.mult
    add = mybir.AluOpType.add

    with ExitStack() as outer:
        const = outer.enter_context(tc.tile_pool(name="const", bufs=1))
        bias_bc = const.tile([P, D], F32)

        attn_pool = outer.enter_context(tc.tile_pool(name="attnout", bufs=DT))
        attn_t = [attn_pool.tile([P, N], MD, tag="attn", name=f"attn{i}")
                  for i in range(DT)]
        outsb = outer.enter_context(tc.tile_pool(name="outsb", bufs=2))
        mm_ps = outer.enter_context(
            tc.tile_pool(name="mmps", bufs=CONFIG["mm_bufs"], space="PSUM"))

        xt_pool = outer.enter_context(tc.tile_pool(name="xT", bufs=1))
        xTall = xt_pool.tile([P, DT * N], MD, tag="xT", name="xTall")

        def xT(dt, lo, sz):
            return xTall[:, dt * N + lo: dt * N + lo + sz]

        vaug_pool = outer.enter_context(tc.tile_pool(name="vaug", bufs=NT))
        vaug = [vaug_pool.tile([P, H * (HD + 1)], VD, tag="vaug",
                               name=f"vaug{i}") for i in range(NT)]

        # qk state lives at outer scope so pair 0 can be computed during
        # phase A (woven into the last v-passes)
        wqk_f = outer.enter_context(tc.tile_pool(name="wqkf", bufs=2))
        qk_pool = outer.enter_context(tc.tile_pool(name="qk", bufs=4))
        wqk = {}     # pair -> list of [P, 2, P] f32r tiles
        qk_t = {}    # pair -> (qT, kT)
        qk_state = {}

        def dma_wqk(pair):
            wf = wqk_f.tile([P, DT, 2, P], MD, tag="wqk", name=f"wqk{pair}")
            for which in range(2):
                src = wqkv_d[:, which * D + pair * P:
                             which * D + (pair + 1) * P].rearrange(
                    "(dc p) e -> p dc e", p=P)
                nc.sync.dma_start(wf[:, :, which, :], src.bitcast(MD))
            wqk[pair] = wf

        def emit_qk_piece(pair, step):
            # 8 steps; each emits 4 dt-matmuls of one accumulation group
            # (which, nch); groups change every 2 steps.
            which, nch = divmod(step // 2, 2)
            sub = step % 2
            if sub == 0:
                if which == 0 and nch == 0:
                    qk_t[pair] = (
                        qk_pool.tile([P, N], MD, tag="qk", name=f"q{pair}"),
                        qk_pool.tile([P, N], MD, tag="qk", name=f"k{pair}"))
                qk_state[pair] = mm_ps.tile([P, 512], F32, tag="mm",
                                            name=f"qp{pair}_{step}")
            qp = qk_state[pair]
            for i in range(4):
                dt = sub * 4 + i
                nc.tensor.matmul(
                    qp[:], wqk[pair][:, dt, which, :],
                    xT(dt, nch * 512, 512),
                    start=(dt == 0), stop=(dt == DT - 1))
            if sub == 1:
                ct = qk_t[pair][which]
                nc.vector.tensor_copy(ct[:, nch * 512:(nch + 1) * 512], qp[:])

        # ---- phase A: x load + transpose; v-pass into v_aug ----
        with ExitStack() as phA:
            scratch = phA.enter_context(tc.tile_pool(name="scratch", bufs=1))
            xload = phA.enter_context(tc.tile_pool(name="xload", bufs=3))
            wv_pool = phA.enter_context(tc.tile_pool(name="wvf", bufs=2))
            tp_ps = phA.enter_context(
                tc.tile_pool(name="tpps", bufs=3, space="PSUM"))

            ident_f = scratch.tile([P, P], F32)
            make_identity(nc, ident_f[:])
            ident_r = scratch.tile([P, P], MD)
            nc.vector.tensor_copy(ident_r[:], ident_f[:])
            ident = ident_r[:]

            def dma_x(nt, chunks=((0, 1024),)):
                xfn = xload.tile([P, D], MD, name="xf")
                for lo, hi in chunks:
                    nc.sync.dma_start(
                        xfn[:, lo:hi],
                        x_d[nt * P:(nt + 1) * P, lo:hi].bitcast(MD))
                return xfn

            def transpose_nt(nt, xfn):
                for half in range(2):
                    tp = tp_ps.tile([P, 512], MD, tag="tp")
                    for q in range(4):
                        dt = half * 4 + q
                        nc.tensor.transpose(tp[:, q * P:(q + 1) * P],
                                            xfn[:, dt * P:(dt + 1) * P], ident)
                    dst = xTall[:].rearrange("p (d n) -> p d n", d=DT)[
                        :, half * 4:(half + 1) * 4, nt * P:(nt + 1) * P]
                    src = tp[:].rearrange("p (d n) -> p d n", d=4)
                    nc.vector.tensor_copy(dst, src)

            def dma_wv(cv):
                wf = wv_pool.tile([P, DT * 512], MD, tag="wv", name=f"wv{cv}")
                srcv = wqkv_d[:, 2 * D + cv * 512: 2 * D + (cv + 1) * 512]
                nc.sync.dma_start(
                    wf[:].rearrange("p (dc w) -> p dc w", dc=DT),
                    srcv.rearrange("(dc p) w -> p dc w", p=P).bitcast(MD))
                return wf

            def v_group(nt, cv):
                vp = mm_ps.tile([P, 512], F32, tag="mm", name=f"vp{nt}_{cv}")
                for dt in range(DT):
                    nc.tensor.matmul(
                        vp[:], xT(dt, nt * P, P),
                        wv[cv][:, dt * 512:(dt + 1) * 512],
                        start=(dt == 0), stop=(dt == DT - 1))
                dstv = vaug[nt][:].rearrange(
                    "p (h e) -> p h e", h=H)[:, 8 * cv:8 * cv + 8, 0:HD]
                srcv = vp[:].rearrange("p (h e) -> p h e", h=8)
                nc.scalar.copy(dstv, srcv)

            # x tiles 0/1 + transposes first, then w_v cv0 (first v-sweep),
            # then remaining x tiles, w_v cv1, w_qk pairs 0/1
            xf0 = dma_x(0, chunks=((0, 128), (128, 512), (512, 1024)))
            transpose_nt(0, xf0)
            xf1 = dma_x(1)
            transpose_nt(1, xf1)
            wv = {0: dma_wv(0)}

            ones_bc = nc.const_aps.tensor(1.0, (P, H), VD)
            for nt in range(NT):
                if nt + 2 < NT:
                    xfn = dma_x(nt + 2)
                    transpose_nt(nt + 2, xfn)
                elif nt == NT - 2:
                    wv[1] = dma_wv(1)
                    dma_wqk(0)
                    dma_wqk(1)
                nc.vector.tensor_copy(
                    vaug[nt][:].rearrange(
                        "p (h e) -> p h e", h=H)[:, :, HD:HD + 1],
                    ones_bc.rearrange("p (h e) -> p h e", e=1))
                v_group(nt, 0)
            for nt in range(NT):
                v_group(nt, 1)
                # weave pair-0 qk into the last two v-passes
                if nt >= NT - 2:
                    step = (nt - (NT - 2)) * 4
                    emit_qk_piece(0, step)
                    emit_qk_piece(0, step + 1)
                    emit_qk_piece(0, step + 2)
                    emit_qk_piece(0, step + 3)

        if phases == "A":
            for nt in range(NT):
                yo = outsb.tile([P, 512], F32, tag="y")
                nc.vector.tensor_copy(yo[:], vaug[nt][:, 0:512])
                nc.sync.dma_start(y_d[nt * P:(nt + 1) * P, 0:512], yo[:])
            return

        # ---- phase B: software-pipelined attention over heads + proj ----
        with ExitStack() as phB:
            p_pool = phB.enter_context(
                tc.tile_pool(name="pT", bufs=CONFIG["p_bufs"]))
            s_ps = phB.enter_context(
                tc.tile_pool(name="sps", bufs=CONFIG["s_bufs"], space="PSUM"))
            rt_pool = phB.enter_context(tc.tile_pool(name="rt", bufs=1))
            bt_pool = phB.enter_context(tc.tile_pool(name="bt", bufs=2))
            wp_f = phB.enter_context(tc.tile_pool(name="wpf", bufs=1))

            p_t = {}     # (h, jt) -> pt tile

            def emit_S(h, jt):
                pair, hh = divmod(h, 2)
                base = HD * hh
                qT, kT = qk_t[pair]
                sp = s_ps.tile([P, N], F32, tag="s")
                for ich in range(2):
                    nc.tensor.matmul(
                        sp[:, ich * 512:(ich + 1) * 512],
                        kT[base:base + HD, jt * P:(jt + 1) * P],
                        qT[base:base + HD, ich * 512:(ich + 1) * 512],
                        start=True, stop=True)
                pt = p_pool.tile([P, N], VD, tag="p", name=f"pT{h}_{jt}")
                nc.scalar.activation(pt[:], sp[:], EXPF, scale=SCALE)
                p_t[(h, jt)] = pt

            def emit_norm(h, pvs):
                pair, hh = divmod(h, 2)
                base = HD * hh
                rt = rt_pool.tile([1, N], F32, tag="rt")
                bt = bt_pool.tile([HD, N], F32, tag="bt")
                for ich in range(2):
                    nc.vector.reciprocal(rt[:, ich * 512:(ich + 1) * 512],
                                         pvs[ich][HD:HD + 1, :])
                nc.gpsimd.partition_broadcast(bt[:], rt[:])
                for ich in range(2):
                    nc.vector.tensor_tensor(
                        attn_t[pair][base:base + HD, ich * 512:(ich + 1) * 512],
                        pvs[ich][0:HD, :], bt[:, ich * 512:(ich + 1) * 512], mul)

            wp = {}

            def dma_wp():
                wf = wp_f.tile([P, DT * D], MD, tag="wp", name="wpall")
                nc.sync.dma_start(
                    wf[:].rearrange("p (cc w) -> p cc w", cc=DT),
                    wproj_d[:, :].rearrange("(cc p) w -> p cc w",
                                            p=P).bitcast(MD))
                wp["all"] = wf

            def proj_group_mms(yp, nt, ec, ccs):
                for cc in ccs:
                    nc.tensor.matmul(
                        yp[:], attn_t[cc][:, nt * P:(nt + 1) * P],
                        wp["all"][:, cc * D + ec * 512: cc * D + ec * 512 + 512],
                        start=(cc == 0), stop=(cc == DT - 1))

            yo_cur = {}

            def proj_group_finish(yp, nt, ec):
                if nt not in yo_cur:
                    yo_cur[nt] = outsb.tile([P, D], F32, tag="y",
                                            name=f"yo{nt}")
                yo = yo_cur[nt]
                nc.vector.tensor_tensor(
                    yo[:, ec * 512:(ec + 1) * 512], yp[:],
                    bias_bc[:, ec * 512:(ec + 1) * 512], add)
                if nt == NT - 1:
                    nc.sync.dma_start(
                        y_d[nt * P:(nt + 1) * P, ec * 512:(ec + 1) * 512],
                        yo[:, ec * 512:(ec + 1) * 512])
                elif ec == 1:
                    nc.sync.dma_start(y_d[nt * P:(nt + 1) * P, :], yo[:])

            for jt in range(NT):
                emit_S(0, jt)

            yp_weave = {}
            for h in range(H):
                pair = h // 2
                if h % 2 == 0 and pair + 2 < H // 2:
                    dma_wqk(pair + 2)
                if h == 1:
                    dma_wp()
                    bstage = rt_pool.tile([1, D], F32, tag="bst")
                    nc.sync.dma_start(
                        bstage[:], bproj_d[:].rearrange("(a f) -> a f", a=1))
                    nc.gpsimd.partition_broadcast(bias_bc[:], bstage[:])
                pvs = [mm_ps.tile([HD + 1, 512], F32, tag="mm",
                                  name=f"pv{h}_{i}") for i in range(2)]
                for jt in range(NT):
                    if h + 1 < H:
                        emit_S(h + 1, jt)
                    if h % 2 == 0 and pair + 1 < H // 2:
                        emit_qk_piece(pair + 1, jt)
                    if h == H - 1 and jt < 2 * NPROJ_WEAVE:
                        # weave partial proj accumulations (cc 0..6) for the
                        # first groups into the drain of the last head
                        g, half = divmod(jt, 2)
                        nt_, ec_ = divmod(g, 2)
                        if half == 0:
                            yp_weave[g] = s_ps.tile([P, 512], F32, tag="s",
                                                    name=f"ypw{g}")
                            proj_group_mms(yp_weave[g], nt_, ec_, range(0, 4))
                        else:
                            proj_group_mms(yp_weave[g], nt_, ec_, range(4, 7))
                    for ich in range(2):
                        nc.tensor.matmul(
                            pvs[ich][:],
                            vaug[jt][:, h * (HD + 1):(h + 1) * (HD + 1)],
                            p_t[(h, jt)][:, ich * 512:(ich + 1) * 512],
                            start=(jt == 0), stop=(jt == NT - 1))
                emit_norm(h, pvs)

            if phases == "AB":
                for cc in range(DT):
                    yo = outsb.tile([P, 512], F32, tag="y")
                    nc.vector.tensor_copy(yo[:], attn_t[cc][:, 0:512])
                    nc.sync.dma_start(y_d[cc * P:(cc + 1) * P, 0:512], yo[:])
                return

            # ---- finish projection ----
            for g in range(NPROJ_WEAVE):
                nt_, ec_ = divmod(g, 2)
                proj_group_mms(yp_weave[g], nt_, ec_, range(7, 8))
                proj_group_finish(yp_weave[g], nt_, ec_)
            for g in range(NPROJ_WEAVE, 2 * NT):
                nt_, ec_ = divmod(g, 2)
                yp = mm_ps.tile([P, 512], F32, tag="mm", name=f"yp{g}")
                proj_group_mms(yp, nt_, ec_, range(DT))
                proj_group_finish(yp, nt_, ec_)


def build_nc(reps=1, phases="full"):
    nc = bacc.Bacc("TRN2", target_bir_lowering=False, debug=False)
    x_d = nc.dram_tensor("x", [N, D], F32, kind="ExternalInput").ap()
    wqkv_d = nc.dram_tensor("w_qkv", [D, 3 * D], F32, kind="ExternalInput").ap()
    wproj_d = nc.dram_tensor("w_proj", [D, D], F32, kind="ExternalInput").ap()
    bproj_d = nc.dram_tensor("b_proj", [D], F32, kind="ExternalInput").ap()
    y_d = nc.dram_tensor("y", [N, D], F32, kind="ExternalOutput").ap()
    with tile.TileContext(nc) as tc:
        for _ in range(reps):
            _build(tc, nc, x_d, wqkv_d, wproj_d, bproj_d, y_d, phases=phases)
    nc.compile()
    return nc


_NC = None


def kernel(x, w_qkv, w_proj, b_proj):
    global _NC
    if _NC is None:
        _NC = build_nc()
    x = np.ascontiguousarray(np.asarray(x, dtype=np.float32))
    w_qkv = np.ascontiguousarray(np.asarray(w_qkv, dtype=np.float32))
    w_proj = np.ascontiguousarray(np.asarray(w_proj, dtype=np.float32))
    b_proj = np.ascontiguousarray(np.asarray(b_proj, dtype=np.float32))
    in_maps = [
        {"x": x[c], "w_qkv": w_qkv, "w_proj": w_proj, "b_proj": b_proj}
        for c in range(NCORES)
    ]
    res = run_bass_kernel_spmd(_NC, in_maps, list(range(NCORES)))
    return np.stack([res.results[c]["y"] for c in range(NCORES)], axis=0)


# revision 27
# speedup vs baseline: 418.6275x; 418.6275x over previous
"""Trainium2 Bass kernel for nn_Attention (dense transformer MHA block).

Reference computation (per batch element b of 8):
    qkv = x @ w_qkv;  q,k,v split into 16 heads of dim 64
    out = softmax(q k^T / 8) v  (per head),  y = out @ w_proj + b_proj

Sharding: pure data-parallel over the batch (B=8 == n_cores). Each core
computes one batch element's full attention with replicated weights; no
collectives. Full inputs in, full outputs out; gather = np.stack.

Per-core dataflow (fp32 weights/x DMA'd directly as f32r -- no convert
copies):
  phase A: per n-tile (2-ahead lookahead), load x, PE-transpose into one
    big xT [d, n] tile (DVE grouped eviction); v-pass matmuls
    (xT-stationary, w_v-moving) packed into v_aug [n, 16*(64+1)] bf16
    tiles with a ones column per head (integrated softmax denominator);
    v_aug evicted on Act. qk matmuls for pair 0 woven into the last two
    v-passes.
  phase B: software-pipelined over heads. Per head-step h: emit S(h+1)
    per j-tile (kT-stationary, qT-moving, K=64) + exp on Act
    (PSUM->SBUF, bf16 out), woven with PV(h) accumulation steps
    (v_aug-stationary, P_T-moving) and, on even steps, the next pair's
    qk matmuls (w-stationary, xT-moving) + DVE evictions. Row 64 of PV
    = softmax denominator -> DVE reciprocal -> Pool partition_broadcast
    -> DVE multiply into attn_T [c, n] (f32r). Projection partial
    accumulations for the first output tiles are woven into the last
    head-step.
  proj: attn_T-stationary @ w_proj-moving; bias added during the
    PSUM->SBUF eviction via a pre-broadcast bias tile (DVE add).
"""

import numpy as np
from contextlib import ExitStack

import concourse.bass as bass
import concourse.bacc as bacc
import concourse.mybir as mybir
from concourse import tile
from concourse.bass_utils import run_bass_kernel_spmd
from concourse.masks import make_identity

F32 = mybir.dt.float32
F32R = mybir.dt.float32r
BF16 = mybir.dt.bfloat16
EXPF = mybir.ActivationFunctionType.Exp

MD = F32R        # matmul operand view dtype for fp32 data
VD = BF16        # v_aug / P_T dtype (PV matmul operands)

CONFIG = {
    "s_bufs": 2,      # S psum tiles in flight ([P, 1024] = 2 banks each)
    "mm_bufs": 4,     # shared matmul-out PSUM pool (1 bank each)
    "p_bufs": 8,      # P_T sbuf tiles (2 heads in flight x 8 j-tiles... bf16)
}

N = 1024          # sequence length (per core)
D = 1024          # model dim
H = 16            # heads
HD = 64           # head dim
SCALE = HD ** -0.5
P = 128           # partitions
NT = N // P       # 8 n-tiles
DT = D // P       # 8 d-chunks
NCORES = 8
NPROJ_WEAVE = 2   # proj output groups partially accumulated in last head-step


def _build(tc, nc, x_d, wqkv_d, wproj_d, bproj_d, y_d, phases="full"):
    mul = mybir.AluOpType.mult
    add = mybir.AluOpType.add

    with ExitStack() as outer:
        const = outer.enter_context(tc.tile_pool(name="const", bufs=1))
        bias_bc = const.tile([P, D], F32)

        attn_pool = outer.enter_context(tc.tile_pool(name="attnout", bufs=DT))
        attn_t = [attn_pool.tile([P, N], MD, tag="attn", name=f"attn{i}")
                  for i in range(DT)]
        outsb = outer.enter_context(tc.tile_pool(name="outsb", bufs=2))
        mm_ps = outer.enter_context(
            tc.tile_pool(name="mmps", bufs=CONFIG["mm_bufs"], space="PSUM"))

        xt_pool = outer.enter_context(tc.tile_pool(name="xT", bufs=1))
        xTall = xt_pool.tile([P, DT * N], MD, tag="xT", name="xTall")

        def xT(dt, lo, sz):
            return xTall[:, dt * N + lo: dt * N + lo + sz]

        vaug_pool = outer.enter_context(tc.tile_pool(name="vaug", bufs=NT))
        vaug = [vaug_pool.tile([P, H * (HD + 1)], VD, tag="vaug",
                               name=f"vaug{i}") for i in range(NT)]

        # qk state lives at outer scope so pair 0 can be computed during
        # phase A (woven into the last v-passes)
        wqk_f = outer.enter_context(tc.tile_pool(name="wqkf", bufs=2))
        qk_pool = outer.enter_context(tc.tile_pool(name="qk", bufs=4))
        wqk = {}     # pair -> list of [P, 2, P] f32r tiles
        qk_t = {}    # pair -> (qT, kT)
        qk_state = {}

        def dma_wqk(pair):
            wf = wqk_f.tile([P, DT, 2, P], MD, tag="wqk", name=f"wqk{pair}")
            for which in range(2):
                src = wqkv_d[:, which * D + pair * P:
                             which * D + (pair + 1) * P].rearrange(
                    "(dc p) e -> p dc e", p=P)
                nc.sync.dma_start(wf[:, :, which, :], src.bitcast(MD))
            wqk[pair] = wf

        def emit_qk_piece(pair, step):
            # 8 steps; each emits 4 dt-matmuls of one accumulation group
            # (which, nch); groups change every 2 steps.
            which, nch = divmod(step // 2, 2)
            sub = step % 2
            if sub == 0:
                if which == 0 and nch == 0:
                    qk_t[pair] = (
                        qk_pool.tile([P, N], MD, tag="qk", name=f"q{pair}"),
                        qk_pool.tile([P, N], MD, tag="qk", name=f"k{pair}"))
                qk_state[pair] = mm_ps.tile([P, 512], F32, tag="mm",
                                            name=f"qp{pair}_{step}")
            qp = qk_state[pair]
            for i in range(4):
                dt = sub * 4 + i
                nc.tensor.matmul(
                    qp[:], wqk[pair][:, dt, which, :],
                    xT(dt, nch * 512, 512),
                    start=(dt == 0), stop=(dt == DT - 1))
            if sub == 1:
                ct = qk_t[pair][which]
                nc.vector.tensor_copy(ct[:, nch * 512:(nch + 1) * 512], qp[:])

        # ---- phase A: x load + transpose; v-pass into v_aug ----
        with ExitStack() as phA:
            scratch = phA.enter_context(tc.tile_pool(name="scratch", bufs=1))
            xload = phA.enter_context(tc.tile_pool(name="xload", bufs=3))
            wv_pool = phA.enter_context(tc.tile_pool(name="wvf", bufs=2))
            tp_ps = phA.enter_context(
                tc.tile_pool(name="tpps", bufs=3, space="PSUM"))

            ident_f = scratch.tile([P, P], F32)
            make_identity(nc, ident_f[:])
            ident_r = scratch.tile([P, P], MD)
            nc.vector.tensor_copy(ident_r[:], ident_f[:])
            ident = ident_r[:]

            def dma_x(nt, chunks=((0, 1024),)):
                xfn = xload.tile([P, D], MD, name="xf")
                for lo, hi in chunks:
                    nc.sync.dma_start(
                        xfn[:, lo:hi],
                        x_d[nt * P:(nt + 1) * P, lo:hi].bitcast(MD))
                return xfn

            def transpose_nt(nt, xfn):
                for half in range(2):
                    tp = tp_ps.tile([P, 512], MD, tag="tp")
                    for q in range(4):
                        dt = half * 4 + q
                        nc.tensor.transpose(tp[:, q * P:(q + 1) * P],
                                            xfn[:, dt * P:(dt + 1) * P], ident)
                    dst = xTall[:].rearrange("p (d n) -> p d n", d=DT)[
                        :, half * 4:(half + 1) * 4, nt * P:(nt + 1) * P]
                    src = tp[:].rearrange("p (d n) -> p d n", d=4)
                    nc.vector.tensor_copy(dst, src)

            def dma_wv(cv):
                wf = wv_pool.tile([P, DT * 512], MD, tag="wv", name=f"wv{cv}")
                hd2 = DT // 2
                for dh in range(2):
                    srcv = wqkv_d[dh * hd2 * P:(dh + 1) * hd2 * P,
                                  2 * D + cv * 512: 2 * D + (cv + 1) * 512]
                    nc.sync.dma_start(
                        wf[:, dh * hd2 * 512:(dh + 1) * hd2 * 512].rearrange(
                            "p (dc w) -> p dc w", dc=hd2),
                        srcv.rearrange("(dc p) w -> p dc w", p=P).bitcast(MD))
                return wf

            def v_group(nt, cv):
                vp = mm_ps.tile([P, 512], F32, tag="mm", name=f"vp{nt}_{cv}")
                for dt in range(DT):
                    nc.tensor.matmul(
                        vp[:], xT(dt, nt * P, P),
                        wv[cv][:, dt * 512:(dt + 1) * 512],
                        start=(dt == 0), stop=(dt == DT - 1))
                dstv = vaug[nt][:].rearrange(
                    "p (h e) -> p h e", h=H)[:, 8 * cv:8 * cv + 8, 0:HD]
                srcv = vp[:].rearrange("p (h e) -> p h e", h=8)
                nc.scalar.copy(dstv, srcv)

            # x tiles 0/1 + transposes first, then w_v cv0 (first v-sweep),
            # then remaining x tiles, w_v cv1, w_qk pairs 0/1
            xf0 = dma_x(0, chunks=((0, 128), (128, 512), (512, 1024)))
            transpose_nt(0, xf0)
            xf1 = dma_x(1, chunks=((0, 512), (512, 1024)))
            transpose_nt(1, xf1)
            wv = {0: dma_wv(0)}

            ones_bc = nc.const_aps.tensor(1.0, (P, H), VD)
            for nt in range(NT):
                if nt + 2 < NT:
                    xfn = dma_x(nt + 2, chunks=((0, 512), (512, 1024)))
                    transpose_nt(nt + 2, xfn)
                elif nt == NT - 2:
                    wv[1] = dma_wv(1)
                    dma_wqk(0)
                    dma_wqk(1)
                nc.vector.tensor_copy(
                    vaug[nt][:].rearrange(
                        "p (h e) -> p h e", h=H)[:, :, HD:HD + 1],
                    ones_bc.rearrange("p (h e) -> p h e", e=1))
                v_group(nt, 0)
            for nt in range(NT):
                v_group(nt, 1)
                # weave pair-0 qk into the last two v-passes
                if nt >= NT - 2:
                    step = (nt - (NT - 2)) * 4
                    emit_qk_piece(0, step)
                    emit_qk_piece(0, step + 1)
                    emit_qk_piece(0, step + 2)
                    emit_qk_piece(0, step + 3)

        if phases == "A":
            for nt in range(NT):
                yo = outsb.tile([P, 512], F32, tag="y")
                nc.vector.tensor_copy(yo[:], vaug[nt][:, 0:512])
                nc.sync.dma_start(y_d[nt * P:(nt + 1) * P, 0:512], yo[:])
            return

        # ---- phase B: software-pipelined attention over heads + proj ----
        with ExitStack() as phB:
            p_pool = phB.enter_context(
                tc.tile_pool(name="pT", bufs=CONFIG["p_bufs"]))
            s_ps = phB.enter_context(
                tc.tile_pool(name="sps", bufs=CONFIG["s_bufs"], space="PSUM"))
            rt_pool = phB.enter_context(tc.tile_pool(name="rt", bufs=1))
            bt_pool = phB.enter_context(tc.tile_pool(name="bt", bufs=2))
            wp_f = phB.enter_context(tc.tile_pool(name="wpf", bufs=1))

            p_t = {}     # (h, jt) -> pt tile

            def emit_S(h, jt):
                pair, hh = divmod(h, 2)
                base = HD * hh
                qT, kT = qk_t[pair]
                sp = s_ps.tile([P, N], F32, tag="s")
                for ich in range(2):
                    nc.tensor.matmul(
                        sp[:, ich * 512:(ich + 1) * 512],
                        kT[base:base + HD, jt * P:(jt + 1) * P],
                        qT[base:base + HD, ich * 512:(ich + 1) * 512],
                        start=True, stop=True)
                pt = p_pool.tile([P, N], VD, tag="p", name=f"pT{h}_{jt}")
                nc.scalar.activation(pt[:], sp[:], EXPF, scale=SCALE)
                p_t[(h, jt)] = pt

            def emit_norm(h, pvs):
                pair, hh = divmod(h, 2)
                base = HD * hh
                rt = rt_pool.tile([1, N], F32, tag="rt")
                bt = bt_pool.tile([HD, N], F32, tag="bt")
                for ich in range(2):
                    nc.vector.reciprocal(rt[:, ich * 512:(ich + 1) * 512],
                                         pvs[ich][HD:HD + 1, :])
                nc.gpsimd.partition_broadcast(bt[:], rt[:])
                for ich in range(2):
                    nc.vector.tensor_tensor(
                        attn_t[pair][base:base + HD, ich * 512:(ich + 1) * 512],
                        pvs[ich][0:HD, :], bt[:, ich * 512:(ich + 1) * 512], mul)

            wp = {}

            def dma_wp():
                wf = wp_f.tile([P, DT * D], MD, tag="wp", name="wpall")
                nc.sync.dma_start(
                    wf[:].rearrange("p (cc w) -> p cc w", cc=DT),
                    wproj_d[:, :].rearrange("(cc p) w -> p cc w",
                                            p=P).bitcast(MD))
                wp["all"] = wf

            def proj_group_mms(yp, nt, ec, ccs):
                for cc in ccs:
                    nc.tensor.matmul(
                        yp[:], attn_t[cc][:, nt * P:(nt + 1) * P],
                        wp["all"][:, cc * D + ec * 512: cc * D + ec * 512 + 512],
                        start=(cc == 0), stop=(cc == DT - 1))

            yo_cur = {}

            def proj_group_finish(yp, nt, ec):
                if nt not in yo_cur:
                    yo_cur[nt] = outsb.tile([P, D], F32, tag="y",
                                            name=f"yo{nt}")
                yo = yo_cur[nt]
                nc.vector.tensor_tensor(
                    yo[:, ec * 512:(ec + 1) * 512], yp[:],
                    bias_bc[:, ec * 512:(ec + 1) * 512], add)
                if nt == NT - 1:
                    nc.sync.dma_start(
                        y_d[nt * P:(nt + 1) * P, ec * 512:(ec + 1) * 512],
                        yo[:, ec * 512:(ec + 1) * 512])
                elif ec == 1:
                    nc.sync.dma_start(y_d[nt * P:(nt + 1) * P, :], yo[:])

            for jt in range(NT):
                emit_S(0, jt)

            yp_weave = {}
            for h in range(H):
                pair = h // 2
                if h % 2 == 0 and pair + 2 < H // 2:
                    dma_wqk(pair + 2)
                if h == 1:
                    dma_wp()
                    bstage = rt_pool.tile([1, D], F32, tag="bst")
                    nc.sync.dma_start(
                        bstage[:], bproj_d[:].rearrange("(a f) -> a f", a=1))
                    nc.gpsimd.partition_broadcast(bias_bc[:], bstage[:])
                pvs = [mm_ps.tile([HD + 1, 512], F32, tag="mm",
                                  name=f"pv{h}_{i}") for i in range(2)]
                for jt in range(NT):
                    if h + 1 < H:
                        emit_S(h + 1, jt)
                    if h % 2 == 0 and pair + 1 < H // 2:
                        emit_qk_piece(pair + 1, jt)
                    if h == H - 1 and jt < 2 * NPROJ_WEAVE:
                        # weave partial proj accumulations (cc 0..6) for the
                        # first groups into the drain of the last head
                        g, half = divmod(jt, 2)
                        nt_, ec_ = divmod(g, 2)
                        if half == 0:
                            yp_weave[g] = s_ps.tile([P, 512], F32, tag="s",
                                                    name=f"ypw{g}")
                            proj_group_mms(yp_weave[g], nt_, ec_, range(0, 4))
                        else:
                            proj_group_mms(yp_weave[g], nt_, ec_, range(4, 7))
                    for ich in range(2):
                        nc.tensor.matmul(
                            pvs[ich][:],
                            vaug[jt][:, h * (HD + 1):(h + 1) * (HD + 1)],
                            p_t[(h, jt)][:, ich * 512:(ich + 1) * 512],
                            start=(jt == 0), stop=(jt == NT - 1))
                emit_norm(h, pvs)

            if phases == "AB":
                for cc in range(DT):
                    yo = outsb.tile([P, 512], F32, tag="y")
                    nc.vector.tensor_copy(yo[:], attn_t[cc][:, 0:512])
                    nc.sync.dma_start(y_d[cc * P:(cc + 1) * P, 0:512], yo[:])
                return

            # ---- finish projection ----
            for g in range(NPROJ_WEAVE):
                nt_, ec_ = divmod(g, 2)
                proj_group_mms(yp_weave[g], nt_, ec_, range(7, 8))
                proj_group_finish(yp_weave[g], nt_, ec_)
            for g in range(NPROJ_WEAVE, 2 * NT):
                nt_, ec_ = divmod(g, 2)
                yp = mm_ps.tile([P, 512], F32, tag="mm", name=f"yp{g}")
                proj_group_mms(yp, nt_, ec_, range(DT))
                proj_group_finish(yp, nt_, ec_)


def build_nc(reps=1, phases="full"):
    nc = bacc.Bacc("TRN2", target_bir_lowering=False, debug=False)
    x_d = nc.dram_tensor("x", [N, D], F32, kind="ExternalInput").ap()
    wqkv_d = nc.dram_tensor("w_qkv", [D, 3 * D], F32, kind="ExternalInput").ap()
    wproj_d = nc.dram_tensor("w_proj", [D, D], F32, kind="ExternalInput").ap()
    bproj_d = nc.dram_tensor("b_proj", [D], F32, kind="ExternalInput").ap()
    y_d = nc.dram_tensor("y", [N, D], F32, kind="ExternalOutput").ap()
    with tile.TileContext(nc) as tc:
        for _ in range(reps):
            _build(tc, nc, x_d, wqkv_d, wproj_d, bproj_d, y_d, phases=phases)
    nc.compile()
    return nc


_NC = None


def kernel(x, w_qkv, w_proj, b_proj):
    global _NC
    if _NC is None:
        _NC = build_nc()
    x = np.ascontiguousarray(np.asarray(x, dtype=np.float32))
    w_qkv = np.ascontiguousarray(np.asarray(w_qkv, dtype=np.float32))
    w_proj = np.ascontiguousarray(np.asarray(w_proj, dtype=np.float32))
    b_proj = np.ascontiguousarray(np.asarray(b_proj, dtype=np.float32))
    in_maps = [
        {"x": x[c], "w_qkv": w_qkv, "w_proj": w_proj, "b_proj": b_proj}
        for c in range(NCORES)
    ]
    res = run_bass_kernel_spmd(_NC, in_maps, list(range(NCORES)))
    return np.stack([res.results[c]["y"] for c in range(NCORES)], axis=0)


# revision 40
# speedup vs baseline: 425.6218x; 1.0167x over previous
"""Trainium2 Bass kernel for nn_Attention (dense transformer MHA block).

Reference computation (per batch element b of 8):
    qkv = x @ w_qkv;  q,k,v split into 16 heads of dim 64
    out = softmax(q k^T / 8) v  (per head),  y = out @ w_proj + b_proj

Sharding: pure data-parallel over the batch (B=8 == n_cores). Each core
computes one batch element's full attention with replicated weights; no
collectives. Full inputs in, full outputs out; gather = np.stack.

Per-core dataflow (fp32 weights/x DMA'd directly as f32r -- no convert
copies):
  phase A: per n-tile (2-ahead lookahead), load x, PE-transpose into one
    big xT [d, n] tile (DVE grouped eviction); v-pass matmuls
    (xT-stationary, w_v-moving) packed into v_aug [n, 16*(64+1)] bf16
    tiles with a ones column per head (integrated softmax denominator);
    v_aug evicted on Act. qk matmuls for pair 0 woven into the last two
    v-passes.
  phase B: software-pipelined over heads. Per head-step h: emit S(h+1)
    per j-tile (kT-stationary, qT-moving, K=64) + exp on Act
    (PSUM->SBUF, bf16 out), woven with PV(h) accumulation steps
    (v_aug-stationary, P_T-moving) and, on even steps, the next pair's
    qk matmuls (w-stationary, xT-moving) + DVE evictions. Row 64 of PV
    = softmax denominator -> DVE reciprocal -> Pool partition_broadcast
    -> DVE multiply into attn_T [c, n] (f32r). Projection partial
    accumulations for the first output tiles are woven into the last
    head-step.
  proj: attn_T-stationary @ w_proj-moving; bias added during the
    PSUM->SBUF eviction via a pre-broadcast bias tile (DVE add).
"""

import numpy as np
from contextlib import ExitStack

import concourse.bass as bass
import concourse.bacc as bacc
import concourse.mybir as mybir
from concourse import tile
from concourse.bass_utils import run_bass_kernel_spmd
from concourse.masks import make_identity

F32 = mybir.dt.float32
F32R = mybir.dt.float32r
BF16 = mybir.dt.bfloat16
EXPF = mybir.ActivationFunctionType.Exp

MD = F32R        # matmul operand view dtype for fp32 data
VD = BF16        # v_aug / P_T dtype (PV matmul operands)

CONFIG = {
    "s_bufs": 2,      # S psum tiles in flight ([P, 1024] = 2 banks each)
    "mm_bufs": 4,     # shared matmul-out PSUM pool (1 bank each)
    "p_bufs": 8,      # P_T sbuf tiles (2 heads in flight x 8 j-tiles... bf16)
}

N = 1024          # sequence length (per core)
D = 1024          # model dim
H = 16            # heads
HD = 64           # head dim
SCALE = HD ** -0.5
P = 128           # partitions
NT = N // P       # 8 n-tiles
DT = D // P       # 8 d-chunks
NCORES = 8
NPROJ_WEAVE = 2   # proj output groups partially accumulated in last head-step


def _build(tc, nc, x_d, wqkv_d, wproj_d, bproj_d, y_d, phases="full"):
    mul = mybir.AluOpType.mult
    add = mybir.AluOpType.add

    with ExitStack() as outer:
        const = outer.enter_context(tc.tile_pool(name="const", bufs=1))
        bias_bc = const.tile([P, D], F32)

        attn_pool = outer.enter_context(tc.tile_pool(name="attnout", bufs=DT))
        attn_t = [attn_pool.tile([P, N], MD, tag="attn", name=f"attn{i}")
                  for i in range(DT)]
        outsb = outer.enter_context(tc.tile_pool(name="outsb", bufs=2))
        mm_ps = outer.enter_context(
            tc.tile_pool(name="mmps", bufs=CONFIG["mm_bufs"], space="PSUM"))

        xt_pool = outer.enter_context(tc.tile_pool(name="xT", bufs=1))
        xTall = xt_pool.tile([P, DT * N], MD, tag="xT", name="xTall")

        def xT(dt, lo, sz):
            return xTall[:, dt * N + lo: dt * N + lo + sz]

        vaug_pool = outer.enter_context(tc.tile_pool(name="vaug", bufs=NT))
        vaug = [vaug_pool.tile([P, H * (HD + 1)], VD, tag="vaug",
                               name=f"vaug{i}") for i in range(NT)]

        # qk state lives at outer scope so pair 0 can be computed during
        # phase A (woven into the last v-passes)
        wqk_f = outer.enter_context(tc.tile_pool(name="wqkf", bufs=2))
        qk_pool = outer.enter_context(tc.tile_pool(name="qk", bufs=4))
        wqk = {}     # pair -> list of [P, 2, P] f32r tiles
        qk_t = {}    # pair -> (qT, kT)
        qk_state = {}

        def dma_wqk(pair):
            wf = wqk_f.tile([P, DT, 2, P], MD, tag="wqk", name=f"wqk{pair}")
            for which in range(2):
                src = wqkv_d[:, which * D + pair * P:
                             which * D + (pair + 1) * P].rearrange(
                    "(dc p) e -> p dc e", p=P)
                nc.sync.dma_start(wf[:, :, which, :], src.bitcast(MD))
            wqk[pair] = wf

        def emit_qk_piece(pair, step):
            # 8 steps; each emits 4 dt-matmuls of one accumulation group
            # (which, nch); groups change every 2 steps.
            which, nch = divmod(step // 2, 2)
            sub = step % 2
            if sub == 0:
                if which == 0 and nch == 0:
                    qk_t[pair] = (
                        qk_pool.tile([P, N], MD, tag="qk", name=f"q{pair}"),
                        qk_pool.tile([P, N], MD, tag="qk", name=f"k{pair}"))
                qk_state[pair] = mm_ps.tile([P, 512], F32, tag="mm",
                                            name=f"qp{pair}_{step}")
            qp = qk_state[pair]
            for i in range(4):
                dt = sub * 4 + i
                nc.tensor.matmul(
                    qp[:], wqk[pair][:, dt, which, :],
                    xT(dt, nch * 512, 512),
                    start=(dt == 0), stop=(dt == DT - 1))
            if sub == 1:
                ct = qk_t[pair][which]
                nc.vector.tensor_copy(ct[:, nch * 512:(nch + 1) * 512], qp[:])

        # ---- phase A: x load + transpose; v-pass into v_aug ----
        with ExitStack() as phA:
            scratch = phA.enter_context(tc.tile_pool(name="scratch", bufs=1))
            xload = phA.enter_context(tc.tile_pool(name="xload", bufs=3))
            wv_pool = phA.enter_context(tc.tile_pool(name="wvf", bufs=2))
            tp_ps = phA.enter_context(
                tc.tile_pool(name="tpps", bufs=3, space="PSUM"))

            ident_f = scratch.tile([P, P], F32)
            make_identity(nc, ident_f[:])
            ident_r = scratch.tile([P, P], MD)
            nc.vector.tensor_copy(ident_r[:], ident_f[:])
            ident = ident_r[:]

            def dma_x(nt, chunks=((0, 1024),)):
                xfn = xload.tile([P, D], MD, name="xf")
                for lo, hi in chunks:
                    nc.sync.dma_start(
                        xfn[:, lo:hi],
                        x_d[nt * P:(nt + 1) * P, lo:hi].bitcast(MD))
                return xfn

            def transpose_nt(nt, xfn):
                for half in range(2):
                    tp = tp_ps.tile([P, 512], MD, tag="tp")
                    for q in range(4):
                        dt = half * 4 + q
                        nc.tensor.transpose(tp[:, q * P:(q + 1) * P],
                                            xfn[:, dt * P:(dt + 1) * P], ident)
                    dst = xTall[:].rearrange("p (d n) -> p d n", d=DT)[
                        :, half * 4:(half + 1) * 4, nt * P:(nt + 1) * P]
                    src = tp[:].rearrange("p (d n) -> p d n", d=4)
                    nc.vector.tensor_copy(dst, src)

            def dma_wv(cv, nsplit=2):
                wf = wv_pool.tile([P, DT * 512], MD, tag="wv", name=f"wv{cv}")
                hd2 = DT // nsplit
                for dh in range(nsplit):
                    srcv = wqkv_d[dh * hd2 * P:(dh + 1) * hd2 * P,
                                  2 * D + cv * 512: 2 * D + (cv + 1) * 512]
                    nc.sync.dma_start(
                        wf[:, dh * hd2 * 512:(dh + 1) * hd2 * 512].rearrange(
                            "p (dc w) -> p dc w", dc=hd2),
                        srcv.rearrange("(dc p) w -> p dc w", p=P).bitcast(MD))
                return wf

            def v_group(nt, cv):
                vp = mm_ps.tile([P, 512], F32, tag="mm", name=f"vp{nt}_{cv}")
                for dt in range(DT):
                    nc.tensor.matmul(
                        vp[:], xT(dt, nt * P, P),
                        wv[cv][:, dt * 512:(dt + 1) * 512],
                        start=(dt == 0), stop=(dt == DT - 1))
                dstv = vaug[nt][:].rearrange(
                    "p (h e) -> p h e", h=H)[:, 8 * cv:8 * cv + 8, 0:HD]
                srcv = vp[:].rearrange("p (h e) -> p h e", h=8)
                nc.scalar.copy(dstv, srcv)

            # x tiles 0/1 + transposes first, then w_v cv0 (first v-sweep),
            # then remaining x tiles, w_v cv1, w_qk pairs 0/1
            xf0 = dma_x(0, chunks=((0, 128), (128, 512), (512, 1024)))
            transpose_nt(0, xf0)
            xf1 = dma_x(1, chunks=((0, 512), (512, 1024)))
            transpose_nt(1, xf1)
            wv = {0: dma_wv(0, nsplit=4)}

            ones_bc = nc.const_aps.tensor(1.0, (P, H), VD)
            for nt in range(NT):
                if nt + 2 < NT:
                    xfn = dma_x(nt + 2, chunks=((0, 512), (512, 1024)))
                    transpose_nt(nt + 2, xfn)
                elif nt == NT - 2:
                    wv[1] = dma_wv(1)
                    dma_wqk(0)
                    dma_wqk(1)
                nc.vector.tensor_copy(
                    vaug[nt][:].rearrange(
                        "p (h e) -> p h e", h=H)[:, :, HD:HD + 1],
                    ones_bc.rearrange("p (h e) -> p h e", e=1))
                v_group(nt, 0)
            for nt in range(NT):
                v_group(nt, 1)
                # weave pair-0 qk into the last two v-passes
                if nt >= NT - 2:
                    step = (nt - (NT - 2)) * 4
                    emit_qk_piece(0, step)
                    emit_qk_piece(0, step + 1)
                    emit_qk_piece(0, step + 2)
                    emit_qk_piece(0, step + 3)

        if phases == "A":
            for nt in range(NT):
                yo = outsb.tile([P, 512], F32, tag="y")
                nc.vector.tensor_copy(yo[:], vaug[nt][:, 0:512])
                nc.sync.dma_start(y_d[nt * P:(nt + 1) * P, 0:512], yo[:])
            return

        # ---- phase B: software-pipelined attention over heads + proj ----
        with ExitStack() as phB:
            p_pool = phB.enter_context(
                tc.tile_pool(name="pT", bufs=CONFIG["p_bufs"]))
            s_ps = phB.enter_context(
                tc.tile_pool(name="sps", bufs=CONFIG["s_bufs"], space="PSUM"))
            rt_pool = phB.enter_context(tc.tile_pool(name="rt", bufs=1))
            bt_pool = phB.enter_context(tc.tile_pool(name="bt", bufs=2))
            wp_f = phB.enter_context(tc.tile_pool(name="wpf", bufs=1))

            p_t = {}     # (h, jt) -> pt tile

            def emit_S(h, jt):
                pair, hh = divmod(h, 2)
                base = HD * hh
                qT, kT = qk_t[pair]
                sp = s_ps.tile([P, N], F32, tag="s")
                for ich in range(2):
                    nc.tensor.matmul(
                        sp[:, ich * 512:(ich + 1) * 512],
                        kT[base:base + HD, jt * P:(jt + 1) * P],
                        qT[base:base + HD, ich * 512:(ich + 1) * 512],
                        start=True, stop=True)
                pt = p_pool.tile([P, N], VD, tag="p", name=f"pT{h}_{jt}")
                nc.scalar.activation(pt[:], sp[:], EXPF, scale=SCALE)
                p_t[(h, jt)] = pt

            def emit_norm(h, pvs):
                pair, hh = divmod(h, 2)
                base = HD * hh
                rt = rt_pool.tile([1, N], F32, tag="rt")
                bt = bt_pool.tile([HD, N], F32, tag="bt")
                for ich in range(2):
                    sl = slice(ich * 512, (ich + 1) * 512)
                    nc.vector.reciprocal(rt[:, sl], pvs[ich][HD:HD + 1, :])
                    nc.gpsimd.partition_broadcast(bt[:, sl], rt[:, sl])
                    nc.vector.tensor_tensor(
                        attn_t[pair][base:base + HD, sl],
                        pvs[ich][0:HD, :], bt[:, sl], mul)

            wp = {}

            def dma_wp():
                wf = wp_f.tile([P, DT * D], MD, tag="wp", name="wpall")
                nc.sync.dma_start(
                    wf[:].rearrange("p (cc w) -> p cc w", cc=DT),
                    wproj_d[:, :].rearrange("(cc p) w -> p cc w",
                                            p=P).bitcast(MD))
                wp["all"] = wf

            def proj_group_mms(yp, nt, ec, ccs):
                for cc in ccs:
                    nc.tensor.matmul(
                        yp[:], attn_t[cc][:, nt * P:(nt + 1) * P],
                        wp["all"][:, cc * D + ec * 512: cc * D + ec * 512 + 512],
                        start=(cc == 0), stop=(cc == DT - 1))

            yo_cur = {}

            def proj_group_finish(yp, nt, ec):
                if nt not in yo_cur:
                    yo_cur[nt] = outsb.tile([P, D], F32, tag="y",
                                            name=f"yo{nt}")
                yo = yo_cur[nt]
                nc.vector.tensor_tensor(
                    yo[:, ec * 512:(ec + 1) * 512], yp[:],
                    bias_bc[:, ec * 512:(ec + 1) * 512], add)
                if nt == NT - 1:
                    nc.sync.dma_start(
                        y_d[nt * P:(nt + 1) * P, ec * 512:(ec + 1) * 512],
                        yo[:, ec * 512:(ec + 1) * 512])
                elif ec == 1:
                    nc.sync.dma_start(y_d[nt * P:(nt + 1) * P, :], yo[:])

            for jt in range(NT):
                emit_S(0, jt)

            yp_weave = {}
            for h in range(H):
                pair = h // 2
                if h % 2 == 0 and pair + 2 < H // 2:
                    dma_wqk(pair + 2)
                if h == 1:
                    dma_wp()
                    bstage = rt_pool.tile([1, D], F32, tag="bst")
                    nc.sync.dma_start(
                        bstage[:], bproj_d[:].rearrange("(a f) -> a f", a=1))
                    nc.gpsimd.partition_broadcast(bias_bc[:], bstage[:])
                pv_pool, pv_tag = (s_ps, "s") if h == H - 1 else (mm_ps, "mm")
                pvs = [pv_pool.tile([HD + 1, 512], F32, tag=pv_tag,
                                    name=f"pv{h}_{i}") for i in range(2)]
                for jt in range(NT):
                    if h + 1 < H:
                        emit_S(h + 1, jt)
                    if h % 2 == 0 and pair + 1 < H // 2:
                        emit_qk_piece(pair + 1, jt)
                    if h == H - 1 and jt < 2 * NPROJ_WEAVE:
                        # weave partial proj accumulations (cc 0..6) for the
                        # first groups into the drain of the last head
                        g, half = divmod(jt, 2)
                        nt_, ec_ = divmod(g, 2)
                        if half == 0:
                            yp_weave[g] = mm_ps.tile([P, 512], F32, tag="mm",
                                                     name=f"ypw{g}")
                            proj_group_mms(yp_weave[g], nt_, ec_, range(0, 4))
                        else:
                            proj_group_mms(yp_weave[g], nt_, ec_, range(4, 7))
                    for ich in range(2):
                        nc.tensor.matmul(
                            pvs[ich][:],
                            vaug[jt][:, h * (HD + 1):(h + 1) * (HD + 1)],
                            p_t[(h, jt)][:, ich * 512:(ich + 1) * 512],
                            start=(jt == 0), stop=(jt == NT - 1))
                emit_norm(h, pvs)

            if phases == "AB":
                for cc in range(DT):
                    yo = outsb.tile([P, 512], F32, tag="y")
                    nc.vector.tensor_copy(yo[:], attn_t[cc][:, 0:512])
                    nc.sync.dma_start(y_d[cc * P:(cc + 1) * P, 0:512], yo[:])
                return

            # ---- finish projection ----
            for g in range(NPROJ_WEAVE):
                nt_, ec_ = divmod(g, 2)
                proj_group_mms(yp_weave[g], nt_, ec_, range(7, 8))
                proj_group_finish(yp_weave[g], nt_, ec_)
            for g in range(NPROJ_WEAVE, 2 * NT):
                nt_, ec_ = divmod(g, 2)
                yp = mm_ps.tile([P, 512], F32, tag="mm", name=f"yp{g}")
                proj_group_mms(yp, nt_, ec_, range(DT))
                proj_group_finish(yp, nt_, ec_)


def build_nc(reps=1, phases="full"):
    nc = bacc.Bacc("TRN2", target_bir_lowering=False, debug=False)
    x_d = nc.dram_tensor("x", [N, D], F32, kind="ExternalInput").ap()
    wqkv_d = nc.dram_tensor("w_qkv", [D, 3 * D], F32, kind="ExternalInput").ap()
    wproj_d = nc.dram_tensor("w_proj", [D, D], F32, kind="ExternalInput").ap()
    bproj_d = nc.dram_tensor("b_proj", [D], F32, kind="ExternalInput").ap()
    y_d = nc.dram_tensor("y", [N, D], F32, kind="ExternalOutput").ap()
    with tile.TileContext(nc) as tc:
        for _ in range(reps):
            _build(tc, nc, x_d, wqkv_d, wproj_d, bproj_d, y_d, phases=phases)
    nc.compile()
    return nc


_NC = None


def kernel(x, w_qkv, w_proj, b_proj):
    global _NC
    if _NC is None:
        _NC = build_nc()
    x = np.ascontiguousarray(np.asarray(x, dtype=np.float32))
    w_qkv = np.ascontiguousarray(np.asarray(w_qkv, dtype=np.float32))
    w_proj = np.ascontiguousarray(np.asarray(w_proj, dtype=np.float32))
    b_proj = np.ascontiguousarray(np.asarray(b_proj, dtype=np.float32))
    in_maps = [
        {"x": x[c], "w_qkv": w_qkv, "w_proj": w_proj, "b_proj": b_proj}
        for c in range(NCORES)
    ]
    res = run_bass_kernel_spmd(_NC, in_maps, list(range(NCORES)))
    return np.stack([res.results[c]["y"] for c in range(NCORES)], axis=0)


# revision 45
# speedup vs baseline: 426.1837x; 1.0013x over previous
"""Trainium2 Bass kernel for nn_Attention (dense transformer MHA block).

Reference computation (per batch element b of 8):
    qkv = x @ w_qkv;  q,k,v split into 16 heads of dim 64
    out = softmax(q k^T / 8) v  (per head),  y = out @ w_proj + b_proj

Sharding: pure data-parallel over the batch (B=8 == n_cores). Each core
computes one batch element's full attention with replicated weights; no
collectives. Full inputs in, full outputs out; gather = np.stack.

Per-core dataflow (fp32 weights/x DMA'd directly as f32r -- no convert
copies):
  phase A: per n-tile (2-ahead lookahead), load x, PE-transpose into one
    big xT [d, n] tile (DVE grouped eviction); v-pass matmuls
    (xT-stationary, w_v-moving) packed into v_aug [n, 16*(64+1)] bf16
    tiles with a ones column per head (integrated softmax denominator);
    v_aug evicted on Act. qk matmuls for pair 0 woven into the last two
    v-passes.
  phase B: software-pipelined over heads. Per head-step h: emit S(h+1)
    per j-tile (kT-stationary, qT-moving, K=64) + exp on Act
    (PSUM->SBUF, bf16 out), woven with PV(h) accumulation steps
    (v_aug-stationary, P_T-moving) and, on even steps, the next pair's
    qk matmuls (w-stationary, xT-moving) + DVE evictions. Row 64 of PV
    = softmax denominator -> DVE reciprocal -> Pool partition_broadcast
    -> DVE multiply into attn_T [c, n] (f32r). Projection partial
    accumulations for the first output tiles are woven into the last
    head-step.
  proj: attn_T-stationary @ w_proj-moving; bias added during the
    PSUM->SBUF eviction via a pre-broadcast bias tile (DVE add).
"""

import numpy as np
from contextlib import ExitStack

import concourse.bass as bass
import concourse.bacc as bacc
import concourse.mybir as mybir
from concourse import tile
from concourse.bass_utils import run_bass_kernel_spmd
from concourse.masks import make_identity

F32 = mybir.dt.float32
F32R = mybir.dt.float32r
BF16 = mybir.dt.bfloat16
EXPF = mybir.ActivationFunctionType.Exp

MD = F32R        # matmul operand view dtype for fp32 data
VD = BF16        # v_aug / P_T dtype (PV matmul operands)

CONFIG = {
    "s_bufs": 2,      # S psum tiles in flight ([P, 1024] = 2 banks each)
    "mm_bufs": 4,     # shared matmul-out PSUM pool (1 bank each)
    "p_bufs": 8,      # P_T sbuf tiles (2 heads in flight x 8 j-tiles... bf16)
}

N = 1024          # sequence length (per core)
D = 1024          # model dim
H = 16            # heads
HD = 64           # head dim
SCALE = HD ** -0.5
P = 128           # partitions
NT = N // P       # 8 n-tiles
DT = D // P       # 8 d-chunks
NCORES = 8
NPROJ_WEAVE = 2   # proj output groups partially accumulated in last head-step


def _build(tc, nc, x_d, wqkv_d, wproj_d, bproj_d, y_d, phases="full"):
    mul = mybir.AluOpType.mult
    add = mybir.AluOpType.add

    with ExitStack() as outer:
        const = outer.enter_context(tc.tile_pool(name="const", bufs=1))
        bias_bc = const.tile([P, D], F32)

        attn_pool = outer.enter_context(tc.tile_pool(name="attnout", bufs=DT))
        attn_t = [attn_pool.tile([P, N], MD, tag="attn", name=f"attn{i}")
                  for i in range(DT)]
        outsb = outer.enter_context(tc.tile_pool(name="outsb", bufs=2))
        mm_ps = outer.enter_context(
            tc.tile_pool(name="mmps", bufs=CONFIG["mm_bufs"], space="PSUM"))

        xt_pool = outer.enter_context(tc.tile_pool(name="xT", bufs=1))
        xTall = xt_pool.tile([P, DT * N], MD, tag="xT", name="xTall")

        def xT(dt, lo, sz):
            return xTall[:, dt * N + lo: dt * N + lo + sz]

        vaug_pool = outer.enter_context(tc.tile_pool(name="vaug", bufs=NT))
        vaug = [vaug_pool.tile([P, H * (HD + 1)], VD, tag="vaug",
                               name=f"vaug{i}") for i in range(NT)]

        # qk state lives at outer scope so pair 0 can be computed during
        # phase A (woven into the last v-passes)
        wqk_f = outer.enter_context(tc.tile_pool(name="wqkf", bufs=2))
        qk_pool = outer.enter_context(tc.tile_pool(name="qk", bufs=4))
        wqk = {}     # pair -> list of [P, 2, P] f32r tiles
        qk_t = {}    # pair -> (qT, kT)
        qk_state = {}

        def dma_wqk(pair):
            wf = wqk_f.tile([P, DT, 2, P], MD, tag="wqk", name=f"wqk{pair}")
            for which in range(2):
                src = wqkv_d[:, which * D + pair * P:
                             which * D + (pair + 1) * P].rearrange(
                    "(dc p) e -> p dc e", p=P)
                nc.sync.dma_start(wf[:, :, which, :], src.bitcast(MD))
            wqk[pair] = wf

        def emit_qk_piece(pair, step):
            # 8 steps; each emits 4 dt-matmuls of one accumulation group
            # (which, nch); groups change every 2 steps.
            which, nch = divmod(step // 2, 2)
            sub = step % 2
            if sub == 0:
                if which == 0 and nch == 0:
                    qk_t[pair] = (
                        qk_pool.tile([P, N], MD, tag="qk", name=f"q{pair}"),
                        qk_pool.tile([P, N], MD, tag="qk", name=f"k{pair}"))
                qk_state[pair] = mm_ps.tile([P, 512], F32, tag="mm",
                                            name=f"qp{pair}_{step}")
            qp = qk_state[pair]
            for i in range(4):
                dt = sub * 4 + i
                nc.tensor.matmul(
                    qp[:], wqk[pair][:, dt, which, :],
                    xT(dt, nch * 512, 512),
                    start=(dt == 0), stop=(dt == DT - 1))
            if sub == 1:
                ct = qk_t[pair][which]
                nc.vector.tensor_copy(ct[:, nch * 512:(nch + 1) * 512], qp[:])

        # ---- phase A: x load + transpose; v-pass into v_aug ----
        with ExitStack() as phA:
            scratch = phA.enter_context(tc.tile_pool(name="scratch", bufs=1))
            xload = phA.enter_context(tc.tile_pool(name="xload", bufs=3))
            wv_pool = phA.enter_context(tc.tile_pool(name="wvf", bufs=2))
            tp_ps = phA.enter_context(
                tc.tile_pool(name="tpps", bufs=3, space="PSUM"))

            ident_f = scratch.tile([P, P], F32)
            make_identity(nc, ident_f[:])
            ident_r = scratch.tile([P, P], MD)
            nc.vector.tensor_copy(ident_r[:], ident_f[:])
            ident = ident_r[:]

            def dma_x(nt, chunks=((0, 1024),)):
                xfn = xload.tile([P, D], MD, name="xf")
                for lo, hi in chunks:
                    nc.sync.dma_start(
                        xfn[:, lo:hi],
                        x_d[nt * P:(nt + 1) * P, lo:hi].bitcast(MD))
                return xfn

            def transpose_nt(nt, xfn):
                for half in range(2):
                    tp = tp_ps.tile([P, 512], MD, tag="tp")
                    for q in range(4):
                        dt = half * 4 + q
                        nc.tensor.transpose(tp[:, q * P:(q + 1) * P],
                                            xfn[:, dt * P:(dt + 1) * P], ident)
                    dst = xTall[:].rearrange("p (d n) -> p d n", d=DT)[
                        :, half * 4:(half + 1) * 4, nt * P:(nt + 1) * P]
                    src = tp[:].rearrange("p (d n) -> p d n", d=4)
                    nc.vector.tensor_copy(dst, src)

            def dma_wv(cv, nsplit=2):
                wf = wv_pool.tile([P, DT * 512], MD, tag="wv", name=f"wv{cv}")
                hd2 = DT // nsplit
                for dh in range(nsplit):
                    srcv = wqkv_d[dh * hd2 * P:(dh + 1) * hd2 * P,
                                  2 * D + cv * 512: 2 * D + (cv + 1) * 512]
                    nc.sync.dma_start(
                        wf[:, dh * hd2 * 512:(dh + 1) * hd2 * 512].rearrange(
                            "p (dc w) -> p dc w", dc=hd2),
                        srcv.rearrange("(dc p) w -> p dc w", p=P).bitcast(MD))
                return wf

            def v_group(nt, cv):
                vp = mm_ps.tile([P, 512], F32, tag="mm", name=f"vp{nt}_{cv}")
                for dt in range(DT):
                    nc.tensor.matmul(
                        vp[:], xT(dt, nt * P, P),
                        wv[cv][:, dt * 512:(dt + 1) * 512],
                        start=(dt == 0), stop=(dt == DT - 1))
                dstv = vaug[nt][:].rearrange(
                    "p (h e) -> p h e", h=H)[:, 8 * cv:8 * cv + 8, 0:HD]
                srcv = vp[:].rearrange("p (h e) -> p h e", h=8)
                nc.scalar.copy(dstv, srcv)

            # x tiles 0/1 + transposes, then w_v cv0, then the remaining x
            # tiles one step ahead of their v-groups (PE stream: T0, T1, v0,
            # T2, v1, T3, v2, ... so each xT eviction hides under the
            # previous v-group), then w_v cv1 + w_qk pairs 0/1
            xf0 = dma_x(0, chunks=((0, 128), (128, 512), (512, 1024)))
            transpose_nt(0, xf0)
            xf1 = dma_x(1, chunks=((0, 512), (512, 1024)))
            transpose_nt(1, xf1)
            wv = {0: dma_wv(0, nsplit=8)}

            ones_bc = nc.const_aps.tensor(1.0, (P, H), VD)

            def ones_fill(nt):
                nc.vector.tensor_copy(
                    vaug[nt][:].rearrange(
                        "p (h e) -> p h e", h=H)[:, :, HD:HD + 1],
                    ones_bc.rearrange("p (h e) -> p h e", e=1))

            ones_fill(0)
            v_group(0, 0)
            for nt in range(1, NT):
                if nt + 1 < NT:
                    xfn = dma_x(nt + 1, chunks=((0, 512), (512, 1024)))
                    transpose_nt(nt + 1, xfn)
                else:
                    wv[1] = dma_wv(1, nsplit=4)
                    dma_wqk(0)
                    dma_wqk(1)
                ones_fill(nt)
                v_group(nt, 0)
            for nt in range(NT):
                v_group(nt, 1)
                # weave pair-0 qk into the last two v-passes
                if nt >= NT - 2:
                    step = (nt - (NT - 2)) * 4
                    emit_qk_piece(0, step)
                    emit_qk_piece(0, step + 1)
                    emit_qk_piece(0, step + 2)
                    emit_qk_piece(0, step + 3)

        if phases == "A":
            for nt in range(NT):
                yo = outsb.tile([P, 512], F32, tag="y")
                nc.vector.tensor_copy(yo[:], vaug[nt][:, 0:512])
                nc.sync.dma_start(y_d[nt * P:(nt + 1) * P, 0:512], yo[:])
            return

        # ---- phase B: software-pipelined attention over heads + proj ----
        with ExitStack() as phB:
            p_pool = phB.enter_context(
                tc.tile_pool(name="pT", bufs=CONFIG["p_bufs"]))
            s_ps = phB.enter_context(
                tc.tile_pool(name="sps", bufs=CONFIG["s_bufs"], space="PSUM"))
            rt_pool = phB.enter_context(tc.tile_pool(name="rt", bufs=1))
            bt_pool = phB.enter_context(tc.tile_pool(name="bt", bufs=2))
            wp_f = phB.enter_context(tc.tile_pool(name="wpf", bufs=1))

            p_t = {}     # (h, jt) -> pt tile

            def emit_S(h, jt):
                pair, hh = divmod(h, 2)
                base = HD * hh
                qT, kT = qk_t[pair]
                sp = s_ps.tile([P, N], F32, tag="s")
                for ich in range(2):
                    nc.tensor.matmul(
                        sp[:, ich * 512:(ich + 1) * 512],
                        kT[base:base + HD, jt * P:(jt + 1) * P],
                        qT[base:base + HD, ich * 512:(ich + 1) * 512],
                        start=True, stop=True)
                pt = p_pool.tile([P, N], VD, tag="p", name=f"pT{h}_{jt}")
                nc.scalar.activation(pt[:], sp[:], EXPF, scale=SCALE)
                p_t[(h, jt)] = pt

            def emit_norm(h, pvs):
                pair, hh = divmod(h, 2)
                base = HD * hh
                rt = rt_pool.tile([1, N], F32, tag="rt")
                bt = bt_pool.tile([HD, N], F32, tag="bt")
                for ich in range(2):
                    sl = slice(ich * 512, (ich + 1) * 512)
                    nc.vector.reciprocal(rt[:, sl], pvs[ich][HD:HD + 1, :])
                    nc.gpsimd.partition_broadcast(bt[:, sl], rt[:, sl])
                    nc.vector.tensor_tensor(
                        attn_t[pair][base:base + HD, sl],
                        pvs[ich][0:HD, :], bt[:, sl], mul)

            wp = {}

            def dma_wp():
                wf = wp_f.tile([P, DT * D], MD, tag="wp", name="wpall")
                nc.sync.dma_start(
                    wf[:].rearrange("p (cc w) -> p cc w", cc=DT),
                    wproj_d[:, :].rearrange("(cc p) w -> p cc w",
                                            p=P).bitcast(MD))
                wp["all"] = wf

            def proj_group_mms(yp, nt, ec, ccs):
                for cc in ccs:
                    nc.tensor.matmul(
                        yp[:], attn_t[cc][:, nt * P:(nt + 1) * P],
                        wp["all"][:, cc * D + ec * 512: cc * D + ec * 512 + 512],
                        start=(cc == 0), stop=(cc == DT - 1))

            yo_cur = {}

            def proj_group_finish(yp, nt, ec):
                if nt not in yo_cur:
                    yo_cur[nt] = outsb.tile([P, D], F32, tag="y",
                                            name=f"yo{nt}")
                yo = yo_cur[nt]
                nc.vector.tensor_tensor(
                    yo[:, ec * 512:(ec + 1) * 512], yp[:],
                    bias_bc[:, ec * 512:(ec + 1) * 512], add)
                if nt == NT - 1:
                    nc.sync.dma_start(
                        y_d[nt * P:(nt + 1) * P, ec * 512:(ec + 1) * 512],
                        yo[:, ec * 512:(ec + 1) * 512])
                elif ec == 1:
                    nc.sync.dma_start(y_d[nt * P:(nt + 1) * P, :], yo[:])

            for jt in range(NT):
                emit_S(0, jt)

            yp_weave = {}
            for h in range(H):
                pair = h // 2
                if h % 2 == 0 and pair + 2 < H // 2:
                    dma_wqk(pair + 2)
                if h == 1:
                    dma_wp()
                    bstage = rt_pool.tile([1, D], F32, tag="bst")
                    nc.sync.dma_start(
                        bstage[:], bproj_d[:].rearrange("(a f) -> a f", a=1))
                    nc.gpsimd.partition_broadcast(bias_bc[:], bstage[:])
                pv_pool, pv_tag = (s_ps, "s") if h == H - 1 else (mm_ps, "mm")
                pvs = [pv_pool.tile([HD + 1, 512], F32, tag=pv_tag,
                                    name=f"pv{h}_{i}") for i in range(2)]
                for jt in range(NT):
                    if h + 1 < H:
                        emit_S(h + 1, jt)
                    if h % 2 == 0 and pair + 1 < H // 2:
                        emit_qk_piece(pair + 1, jt)
                    if h == H - 1 and jt < 2 * NPROJ_WEAVE:
                        # weave partial proj accumulations (cc 0..6) for the
                        # first groups into the drain of the last head
                        g, half = divmod(jt, 2)
                        nt_, ec_ = divmod(g, 2)
                        if half == 0:
                            yp_weave[g] = mm_ps.tile([P, 512], F32, tag="mm",
                                                     name=f"ypw{g}")
                            proj_group_mms(yp_weave[g], nt_, ec_, range(0, 4))
                        else:
                            proj_group_mms(yp_weave[g], nt_, ec_, range(4, 7))
                    for ich in range(2):
                        nc.tensor.matmul(
                            pvs[ich][:],
                            vaug[jt][:, h * (HD + 1):(h + 1) * (HD + 1)],
                            p_t[(h, jt)][:, ich * 512:(ich + 1) * 512],
                            start=(jt == 0), stop=(jt == NT - 1))
                emit_norm(h, pvs)

            if phases == "AB":
                for cc in range(DT):
                    yo = outsb.tile([P, 512], F32, tag="y")
                    nc.vector.tensor_copy(yo[:], attn_t[cc][:, 0:512])
                    nc.sync.dma_start(y_d[cc * P:(cc + 1) * P, 0:512], yo[:])
                return

            # ---- finish projection ----
            for g in range(NPROJ_WEAVE):
                nt_, ec_ = divmod(g, 2)
                proj_group_mms(yp_weave[g], nt_, ec_, range(7, 8))
                proj_group_finish(yp_weave[g], nt_, ec_)
            for g in range(NPROJ_WEAVE, 2 * NT):
                nt_, ec_ = divmod(g, 2)
                yp = mm_ps.tile([P, 512], F32, tag="mm", name=f"yp{g}")
                proj_group_mms(yp, nt_, ec_, range(DT))
                proj_group_finish(yp, nt_, ec_)


def build_nc(reps=1, phases="full"):
    nc = bacc.Bacc("TRN2", target_bir_lowering=False, debug=False)
    x_d = nc.dram_tensor("x", [N, D], F32, kind="ExternalInput").ap()
    wqkv_d = nc.dram_tensor("w_qkv", [D, 3 * D], F32, kind="ExternalInput").ap()
    wproj_d = nc.dram_tensor("w_proj", [D, D], F32, kind="ExternalInput").ap()
    bproj_d = nc.dram_tensor("b_proj", [D], F32, kind="ExternalInput").ap()
    y_d = nc.dram_tensor("y", [N, D], F32, kind="ExternalOutput").ap()
    with tile.TileContext(nc) as tc:
        for _ in range(reps):
            _build(tc, nc, x_d, wqkv_d, wproj_d, bproj_d, y_d, phases=phases)
    nc.compile()
    return nc


_NC = None


def kernel(x, w_qkv, w_proj, b_proj):
    global _NC
    if _NC is None:
        _NC = build_nc()
    x = np.ascontiguousarray(np.asarray(x, dtype=np.float32))
    w_qkv = np.ascontiguousarray(np.asarray(w_qkv, dtype=np.float32))
    w_proj = np.ascontiguousarray(np.asarray(w_proj, dtype=np.float32))
    b_proj = np.ascontiguousarray(np.asarray(b_proj, dtype=np.float32))
    in_maps = [
        {"x": x[c], "w_qkv": w_qkv, "w_proj": w_proj, "b_proj": b_proj}
        for c in range(NCORES)
    ]
    res = run_bass_kernel_spmd(_NC, in_maps, list(range(NCORES)))
    return np.stack([res.results[c]["y"] for c in range(NCORES)], axis=0)


# revision 51
# speedup vs baseline: 428.2743x; 1.0049x over previous
"""Trainium2 Bass kernel for nn_Attention (dense transformer MHA block).

Reference computation (per batch element b of 8):
    qkv = x @ w_qkv;  q,k,v split into 16 heads of dim 64
    out = softmax(q k^T / 8) v  (per head),  y = out @ w_proj + b_proj

Sharding: pure data-parallel over the batch (B=8 == n_cores). Each core
computes one batch element's full attention with replicated weights; no
collectives. Full inputs in, full outputs out; gather = np.stack.

Per-core dataflow (fp32 weights/x DMA'd directly as f32r -- no convert
copies):
  phase A: per n-tile (2-ahead lookahead), load x, PE-transpose into one
    big xT [d, n] tile (DVE grouped eviction); v-pass matmuls
    (xT-stationary, w_v-moving) packed into v_aug [n, 16*(64+1)] bf16
    tiles with a ones column per head (integrated softmax denominator);
    v_aug evicted on Act. qk matmuls for pair 0 woven into the last two
    v-passes.
  phase B: software-pipelined over heads. Per head-step h: emit S(h+1)
    per j-tile (kT-stationary, qT-moving, K=64) + exp on Act
    (PSUM->SBUF, bf16 out), woven with PV(h) accumulation steps
    (v_aug-stationary, P_T-moving) and, on even steps, the next pair's
    qk matmuls (w-stationary, xT-moving) + DVE evictions. Row 64 of PV
    = softmax denominator -> DVE reciprocal -> Pool partition_broadcast
    -> DVE multiply into attn_T [c, n] (f32r). Projection partial
    accumulations for the first output tiles are woven into the last
    head-step.
  proj: attn_T-stationary @ w_proj-moving; bias added during the
    PSUM->SBUF eviction via a pre-broadcast bias tile (DVE add).
"""

import numpy as np
from contextlib import ExitStack

import concourse.bass as bass
import concourse.bacc as bacc
import concourse.mybir as mybir
from concourse import tile
from concourse.bass_utils import run_bass_kernel_spmd
from concourse.masks import make_identity

F32 = mybir.dt.float32
F32R = mybir.dt.float32r
BF16 = mybir.dt.bfloat16
EXPF = mybir.ActivationFunctionType.Exp

MD = F32R        # matmul operand view dtype for fp32 data
VD = BF16        # v_aug / P_T dtype (PV matmul operands)

CONFIG = {
    "s_bufs": 2,      # S psum tiles in flight ([P, 1024] = 2 banks each)
    "mm_bufs": 4,     # shared matmul-out PSUM pool (1 bank each)
    "p_bufs": 8,      # P_T sbuf tiles (2 heads in flight x 8 j-tiles... bf16)
}

N = 1024          # sequence length (per core)
D = 1024          # model dim
H = 16            # heads
HD = 64           # head dim
SCALE = HD ** -0.5
P = 128           # partitions
NT = N // P       # 8 n-tiles
DT = D // P       # 8 d-chunks
NCORES = 8
NPROJ_WEAVE = 2   # proj output groups partially accumulated in last head-step


def _build(tc, nc, x_d, wqkv_d, wproj_d, bproj_d, y_d, phases="full"):
    mul = mybir.AluOpType.mult
    add = mybir.AluOpType.add

    with ExitStack() as outer:
        const = outer.enter_context(tc.tile_pool(name="const", bufs=1))
        bias_bc = const.tile([P, D], F32)

        attn_pool = outer.enter_context(tc.tile_pool(name="attnout", bufs=DT))
        attn_t = [attn_pool.tile([P, N], MD, tag="attn", name=f"attn{i}")
                  for i in range(DT)]
        outsb = outer.enter_context(tc.tile_pool(name="outsb", bufs=2))
        mm_ps = outer.enter_context(
            tc.tile_pool(name="mmps", bufs=CONFIG["mm_bufs"], space="PSUM"))

        xt_pool = outer.enter_context(tc.tile_pool(name="xT", bufs=1))
        xTall = xt_pool.tile([P, DT * N], MD, tag="xT", name="xTall")

        def xT(dt, lo, sz):
            return xTall[:, dt * N + lo: dt * N + lo + sz]

        vaug_pool = outer.enter_context(tc.tile_pool(name="vaug", bufs=NT))
        vaug = [vaug_pool.tile([P, H * (HD + 1)], VD, tag="vaug",
                               name=f"vaug{i}") for i in range(NT)]

        # qk state lives at outer scope so pair 0 can be computed during
        # phase A (woven into the last v-passes)
        wqk_f = outer.enter_context(tc.tile_pool(name="wqkf", bufs=2))
        qk_pool = outer.enter_context(tc.tile_pool(name="qk", bufs=4))
        wqk = {}     # pair -> list of [P, 2, P] f32r tiles
        qk_t = {}    # pair -> (qT, kT)
        qk_state = {}

        def dma_wqk(pair):
            wf = wqk_f.tile([P, DT, 2, P], MD, tag="wqk", name=f"wqk{pair}")
            for which in range(2):
                src = wqkv_d[:, which * D + pair * P:
                             which * D + (pair + 1) * P].rearrange(
                    "(dc p) e -> p dc e", p=P)
                nc.sync.dma_start(wf[:, :, which, :], src.bitcast(MD))
            wqk[pair] = wf

        def emit_qk_piece(pair, step):
            # 8 steps; each emits 4 dt-matmuls of one accumulation group
            # (which, nch); groups change every 2 steps.
            which, nch = divmod(step // 2, 2)
            sub = step % 2
            if sub == 0:
                if which == 0 and nch == 0:
                    qk_t[pair] = (
                        qk_pool.tile([P, N], MD, tag="qk", name=f"q{pair}"),
                        qk_pool.tile([P, N], MD, tag="qk", name=f"k{pair}"))
                qk_state[pair] = mm_ps.tile([P, 512], F32, tag="mm",
                                            name=f"qp{pair}_{step}")
            qp = qk_state[pair]
            for i in range(4):
                dt = sub * 4 + i
                nc.tensor.matmul(
                    qp[:], wqk[pair][:, dt, which, :],
                    xT(dt, nch * 512, 512),
                    start=(dt == 0), stop=(dt == DT - 1))
            if sub == 1:
                ct = qk_t[pair][which]
                nc.vector.tensor_copy(ct[:, nch * 512:(nch + 1) * 512], qp[:])

        # ---- phase A: x load + transpose; v-pass into v_aug ----
        with ExitStack() as phA:
            scratch = phA.enter_context(tc.tile_pool(name="scratch", bufs=1))
            xload = phA.enter_context(tc.tile_pool(name="xload", bufs=3))
            wv_pool = phA.enter_context(tc.tile_pool(name="wvf", bufs=2))
            tp_ps = phA.enter_context(
                tc.tile_pool(name="tpps", bufs=3, space="PSUM"))

            ident_f = scratch.tile([P, P], F32)
            make_identity(nc, ident_f[:])
            ident_r = scratch.tile([P, P], MD)
            nc.vector.tensor_copy(ident_r[:], ident_f[:])
            ident = ident_r[:]

            def dma_x(nt, chunks=((0, 1024),)):
                xfn = xload.tile([P, D], MD, name="xf")
                for lo, hi in chunks:
                    nc.sync.dma_start(
                        xfn[:, lo:hi],
                        x_d[nt * P:(nt + 1) * P, lo:hi].bitcast(MD))
                return xfn

            def transpose_nt(nt, xfn):
                for half in range(2):
                    tp = tp_ps.tile([P, 512], MD, tag="tp")
                    for q in range(4):
                        dt = half * 4 + q
                        nc.tensor.transpose(tp[:, q * P:(q + 1) * P],
                                            xfn[:, dt * P:(dt + 1) * P], ident)
                    dst = xTall[:].rearrange("p (d n) -> p d n", d=DT)[
                        :, half * 4:(half + 1) * 4, nt * P:(nt + 1) * P]
                    src = tp[:].rearrange("p (d n) -> p d n", d=4)
                    nc.vector.tensor_copy(dst, src)

            def dma_wv_chunk(wf, cv, dh, nsplit):
                hd2 = DT // nsplit
                srcv = wqkv_d[dh * hd2 * P:(dh + 1) * hd2 * P,
                              2 * D + cv * 512: 2 * D + (cv + 1) * 512]
                nc.sync.dma_start(
                    wf[:, dh * hd2 * 512:(dh + 1) * hd2 * 512].rearrange(
                        "p (dc w) -> p dc w", dc=hd2),
                    srcv.rearrange("(dc p) w -> p dc w", p=P).bitcast(MD))

            def dma_wv(cv, nsplit=2):
                wf = wv_pool.tile([P, DT * 512], MD, tag="wv", name=f"wv{cv}")
                for dh in range(nsplit):
                    dma_wv_chunk(wf, cv, dh, nsplit)
                return wf

            def v_group(nt, cv):
                vp = mm_ps.tile([P, 512], F32, tag="mm", name=f"vp{nt}_{cv}")
                for dt in range(DT):
                    nc.tensor.matmul(
                        vp[:], xT(dt, nt * P, P),
                        wv[cv][:, dt * 512:(dt + 1) * 512],
                        start=(dt == 0), stop=(dt == DT - 1))
                dstv = vaug[nt][:].rearrange(
                    "p (h e) -> p h e", h=H)[:, 8 * cv:8 * cv + 8, 0:HD]
                srcv = vp[:].rearrange("p (h e) -> p h e", h=8)
                nc.scalar.copy(dstv, srcv)

            # x tiles 0/1 + transposes, then w_v cv0, then the remaining x
            # tiles one step ahead of their v-groups (PE stream: T0, T1, v0,
            # T2, v1, T3, v2, ... so each xT eviction hides under the
            # previous v-group), then w_v cv1 + w_qk pairs 0/1
            xf0 = dma_x(0, chunks=((0, 128), (128, 512), (512, 1024)))
            transpose_nt(0, xf0)
            xf1 = dma_x(1, chunks=((0, 512), (512, 1024)))
            transpose_nt(1, xf1)
            # wv0 chunks interleaved with x2/x3 so T2/T3 can fill the
            # DMA-paced start of the first v-sweep
            wv = {0: wv_pool.tile([P, DT * 512], MD, tag="wv", name="wv0")}
            xfs = {}
            dma_wv_chunk(wv[0], 0, 0, 8)
            dma_wv_chunk(wv[0], 0, 1, 8)
            xfs[2] = dma_x(2, chunks=((0, 512), (512, 1024)))
            dma_wv_chunk(wv[0], 0, 2, 8)
            dma_wv_chunk(wv[0], 0, 3, 8)
            xfs[3] = dma_x(3, chunks=((0, 512), (512, 1024)))
            for dh in range(4, 8):
                dma_wv_chunk(wv[0], 0, dh, 8)

            ones_bc = nc.const_aps.tensor(1.0, (P, H), VD)

            def ones_fill(nt):
                nc.vector.tensor_copy(
                    vaug[nt][:].rearrange(
                        "p (h e) -> p h e", h=H)[:, :, HD:HD + 1],
                    ones_bc.rearrange("p (h e) -> p h e", e=1))

            ones_fill(0)
            v_group(0, 0)
            for nt in range(1, NT):
                if nt + 1 < NT:
                    xfn = xfs.get(nt + 1)
                    if xfn is None:
                        xfn = dma_x(nt + 1, chunks=((0, 512), (512, 1024)))
                    transpose_nt(nt + 1, xfn)
                else:
                    wv[1] = dma_wv(1, nsplit=4)
                    dma_wqk(0)
                    dma_wqk(1)
                ones_fill(nt)
                v_group(nt, 0)
            for nt in range(NT):
                v_group(nt, 1)
                # weave pair-0 qk into the last two v-passes
                if nt >= NT - 2:
                    step = (nt - (NT - 2)) * 4
                    emit_qk_piece(0, step)
                    emit_qk_piece(0, step + 1)
                    emit_qk_piece(0, step + 2)
                    emit_qk_piece(0, step + 3)

        if phases == "A":
            for nt in range(NT):
                yo = outsb.tile([P, 512], F32, tag="y")
                nc.vector.tensor_copy(yo[:], vaug[nt][:, 0:512])
                nc.sync.dma_start(y_d[nt * P:(nt + 1) * P, 0:512], yo[:])
            return

        # ---- phase B: software-pipelined attention over heads + proj ----
        with ExitStack() as phB:
            p_pool = phB.enter_context(
                tc.tile_pool(name="pT", bufs=CONFIG["p_bufs"]))
            s_ps = phB.enter_context(
                tc.tile_pool(name="sps", bufs=CONFIG["s_bufs"], space="PSUM"))
            rt_pool = phB.enter_context(tc.tile_pool(name="rt", bufs=1))
            bt_pool = phB.enter_context(tc.tile_pool(name="bt", bufs=2))
            wp_f = phB.enter_context(tc.tile_pool(name="wpf", bufs=1))

            p_t = {}     # (h, jt) -> pt tile

            def emit_S(h, jt):
                pair, hh = divmod(h, 2)
                base = HD * hh
                qT, kT = qk_t[pair]
                sp = s_ps.tile([P, N], F32, tag="s")
                for ich in range(2):
                    nc.tensor.matmul(
                        sp[:, ich * 512:(ich + 1) * 512],
                        kT[base:base + HD, jt * P:(jt + 1) * P],
                        qT[base:base + HD, ich * 512:(ich + 1) * 512],
                        start=True, stop=True)
                pt = p_pool.tile([P, N], VD, tag="p", name=f"pT{h}_{jt}")
                nc.scalar.activation(pt[:], sp[:], EXPF, scale=SCALE)
                p_t[(h, jt)] = pt

            def emit_norm(h, pvs):
                pair, hh = divmod(h, 2)
                base = HD * hh
                rt = rt_pool.tile([1, N], F32, tag="rt")
                bt = bt_pool.tile([HD, N], F32, tag="bt")
                for ich in range(2):
                    sl = slice(ich * 512, (ich + 1) * 512)
                    nc.vector.reciprocal(rt[:, sl], pvs[ich][HD:HD + 1, :])
                    nc.gpsimd.partition_broadcast(bt[:, sl], rt[:, sl])
                    nc.vector.tensor_tensor(
                        attn_t[pair][base:base + HD, sl],
                        pvs[ich][0:HD, :], bt[:, sl], mul)

            wp = {}

            def dma_wp():
                wf = wp_f.tile([P, DT * D], MD, tag="wp", name="wpall")
                nc.sync.dma_start(
                    wf[:].rearrange("p (cc w) -> p cc w", cc=DT),
                    wproj_d[:, :].rearrange("(cc p) w -> p cc w",
                                            p=P).bitcast(MD))
                wp["all"] = wf

            def proj_group_mms(yp, nt, ec, ccs):
                for cc in ccs:
                    nc.tensor.matmul(
                        yp[:], attn_t[cc][:, nt * P:(nt + 1) * P],
                        wp["all"][:, cc * D + ec * 512: cc * D + ec * 512 + 512],
                        start=(cc == 0), stop=(cc == DT - 1))

            yo_cur = {}

            def proj_group_finish(yp, nt, ec):
                if nt not in yo_cur:
                    yo_cur[nt] = outsb.tile([P, D], F32, tag="y",
                                            name=f"yo{nt}")
                yo = yo_cur[nt]
                nc.vector.tensor_tensor(
                    yo[:, ec * 512:(ec + 1) * 512], yp[:],
                    bias_bc[:, ec * 512:(ec + 1) * 512], add)
                if nt == NT - 1:
                    nc.sync.dma_start(
                        y_d[nt * P:(nt + 1) * P, ec * 512:(ec + 1) * 512],
                        yo[:, ec * 512:(ec + 1) * 512])
                elif ec == 1:
                    nc.sync.dma_start(y_d[nt * P:(nt + 1) * P, :], yo[:])

            for jt in range(NT):
                emit_S(0, jt)

            yp_weave = {}
            for h in range(H):
                pair = h // 2
                if h % 2 == 0 and pair + 2 < H // 2:
                    dma_wqk(pair + 2)
                if h == 1:
                    dma_wp()
                    bstage = rt_pool.tile([1, D], F32, tag="bst")
                    nc.sync.dma_start(
                        bstage[:], bproj_d[:].rearrange("(a f) -> a f", a=1))
                    nc.gpsimd.partition_broadcast(bias_bc[:], bstage[:])
                pv_pool, pv_tag = (s_ps, "s") if h == H - 1 else (mm_ps, "mm")
                pvs = [pv_pool.tile([HD + 1, 512], F32, tag=pv_tag,
                                    name=f"pv{h}_{i}") for i in range(2)]
                for jt in range(NT):
                    if h + 1 < H:
                        emit_S(h + 1, jt)
                    if h % 2 == 0 and pair + 1 < H // 2:
                        emit_qk_piece(pair + 1, jt)
                    if h == H - 1 and jt < 2 * NPROJ_WEAVE:
                        # weave partial proj accumulations (cc 0..6) for the
                        # first groups into the drain of the last head
                        g, half = divmod(jt, 2)
                        nt_, ec_ = divmod(g, 2)
                        if half == 0:
                            yp_weave[g] = mm_ps.tile([P, 512], F32, tag="mm",
                                                     name=f"ypw{g}")
                            proj_group_mms(yp_weave[g], nt_, ec_, range(0, 4))
                        else:
                            proj_group_mms(yp_weave[g], nt_, ec_, range(4, 7))
                    for ich in range(2):
                        nc.tensor.matmul(
                            pvs[ich][:],
                            vaug[jt][:, h * (HD + 1):(h + 1) * (HD + 1)],
                            p_t[(h, jt)][:, ich * 512:(ich + 1) * 512],
                            start=(jt == 0), stop=(jt == NT - 1))
                emit_norm(h, pvs)

            if phases == "AB":
                for cc in range(DT):
                    yo = outsb.tile([P, 512], F32, tag="y")
                    nc.vector.tensor_copy(yo[:], attn_t[cc][:, 0:512])
                    nc.sync.dma_start(y_d[cc * P:(cc + 1) * P, 0:512], yo[:])
                return

            # ---- finish projection ----
            for g in range(NPROJ_WEAVE):
                nt_, ec_ = divmod(g, 2)
                proj_group_mms(yp_weave[g], nt_, ec_, range(7, 8))
                proj_group_finish(yp_weave[g], nt_, ec_)
            for g in range(NPROJ_WEAVE, 2 * NT):
                nt_, ec_ = divmod(g, 2)
                yp = mm_ps.tile([P, 512], F32, tag="mm", name=f"yp{g}")
                proj_group_mms(yp, nt_, ec_, range(DT))
                proj_group_finish(yp, nt_, ec_)


def build_nc(reps=1, phases="full"):
    nc = bacc.Bacc("TRN2", target_bir_lowering=False, debug=False)
    x_d = nc.dram_tensor("x", [N, D], F32, kind="ExternalInput").ap()
    wqkv_d = nc.dram_tensor("w_qkv", [D, 3 * D], F32, kind="ExternalInput").ap()
    wproj_d = nc.dram_tensor("w_proj", [D, D], F32, kind="ExternalInput").ap()
    bproj_d = nc.dram_tensor("b_proj", [D], F32, kind="ExternalInput").ap()
    y_d = nc.dram_tensor("y", [N, D], F32, kind="ExternalOutput").ap()
    with tile.TileContext(nc) as tc:
        for _ in range(reps):
            _build(tc, nc, x_d, wqkv_d, wproj_d, bproj_d, y_d, phases=phases)
    nc.compile()
    return nc


_NC = None


def kernel(x, w_qkv, w_proj, b_proj):
    global _NC
    if _NC is None:
        _NC = build_nc()
    x = np.ascontiguousarray(np.asarray(x, dtype=np.float32))
    w_qkv = np.ascontiguousarray(np.asarray(w_qkv, dtype=np.float32))
    w_proj = np.ascontiguousarray(np.asarray(w_proj, dtype=np.float32))
    b_proj = np.ascontiguousarray(np.asarray(b_proj, dtype=np.float32))
    in_maps = [
        {"x": x[c], "w_qkv": w_qkv, "w_proj": w_proj, "b_proj": b_proj}
        for c in range(NCORES)
    ]
    res = run_bass_kernel_spmd(_NC, in_maps, list(range(NCORES)))
    return np.stack([res.results[c]["y"] for c in range(NCORES)], axis=0)
